# revision 59
# baseline (speedup 1.0000x reference)
"""BinConv (binarize-both-operands 3x3 conv, stride 1, pad 1) on 8 trn2 cores.

Strategy: data-parallel over batch (4 images per core), weights replicated.

Per-core device kernel (memory-roofline oriented; tensor-bound at ~111us of
fp8 DoubleRow matmul stream):
  - x is uploaded as fp8e4 (host dtype cast; -0.0 bytes are remapped to a
    small negative normal so the cast is SIGN-exact -> device binarization
    matches the fp32 reference bit-for-bit). 4x less input HBM traffic.
  - On device x is binarized with one exact DVE op (is_ge 0.0, subtract 0.5)
    -> {-0.5, +0.5} in fp8e4 into a fully zero-padded buffer (114x114 per
    image), so each 3x3 tap is a strided-AP matmul with no edge corrections.
  - Weights arrive host-transposed as [c_in, 10, c_out] fp32 (slot 9 is an
    unused tap-8 copy) and binarize on device to {-0.5, +0.5}. Each output
    tile accumulates 5 matmul passes: 4 fp8 DoubleRow tap-pairs + 1 single
    (tap 8). Products are exactly +-0.25; PSUM fp32 accumulation is exact,
    and the x4 rescale rides free on the ACT drain (scale=4).
  - PSUM -> SBUF in fp16 via ACT activation(Identity, bias, scale=4);
    outputs are integers <= 1152 (exact in fp16) plus a ~0.01 bias. DMA out
    as fp16 (2x less output HBM traffic), upcast to fp32 on host.
  - Scheduling: image i+1's chunks/borders are emitted before image i's
    compute so no engine FIFO head-of-line blocks the pipeline; output
    doorbells ride on gpsimd; the first/last tile groups taper ([1,1,2,3] /
    [4,2,1]) to shorten the kernel head and tail.
"""

import os
import sys

import numpy as np

for _p in ("/opt/trn_rl_repo", "/opt/pypackages"):
    if _p not in sys.path and os.path.isdir(_p):
        sys.path.append(_p)

import ml_dtypes  # noqa: E402

from concourse import bacc, bass, mybir, tile  # noqa: E402
from concourse.ap import AP  # noqa: E402
from concourse.bass_utils import run_bass_kernel_spmd  # noqa: E402

F32 = mybir.dt.float32
F16 = mybir.dt.float16
F8 = mybir.dt.float8e4
ALU = mybir.AluOpType
ACTF = mybir.ActivationFunctionType

N_CORES = 8
P = 128  # C_in == C_out == partitions
H = W = 112
HWIMG = H * W  # 12544
IMGS = 4  # images per core
QROWS = 28  # rows per DMA chunk / output quarter
CHUNK = QROWS * W  # 3136
NTILE = 448  # matmul free dim (4 output rows), one PSUM bank
TROWS = NTILE // W  # 4
TILES_PER_CHUNK = CHUNK // NTILE  # 7
RS = W + 2  # padded row stride (112 data + zero col each side)
TSIZE = (H + 2) * RS  # 114*114 = 12996

# tap t = (kh, kw); for the output tile starting at row r0, tap t reads the
# padded buffer at base (r0+kh)*RS + kw with free dims [TROWS @ RS, W @ 1]
OFF = [(t // 3) * RS + (t % 3) for t in range(9)]

# matmul variant: "C" = 4 DoubleRow pairs + 1 single; "Z" = 5 DoubleRow
# pairs, the last being tap 8 twice at half weight with pair stride 0.
VARIANT = os.environ.get("BINCONV_VARIANT", "C")


def _rhs_ap(T: bass.AP, base: int, pair_d: int | None) -> bass.AP:
    """Strided tap view of the padded image buffer: [P, (2,) TROWS, W]."""
    pstride = list(T.ap[0])
    dims = [pstride]
    if pair_d is not None:
        dims.append([pair_d, 2])
    dims += [[RS, TROWS], [1, W]]
    return AP(T.tensor, base, dims)


def _emit_main_matmuls(nc, ps_list, wb2, T, r0_list, variant):
    """Accumulate all taps into each PSUM tile (one per output row-group).

    Loops weight-sets outermost so consecutive matmuls share the stationary
    operand (amortizes LDWEIGHTS across the tiles in the group).
    """
    dr = mybir.MatmulPerfMode.DoubleRow
    if variant in ("C", "P"):
        groups = [((2 * p, 2 * p + 1), True) for p in range(4)] + [((8,), False)]
    else:
        raise ValueError(variant)
    # variant P: run the odd tap-8 single in DoublePixel mode (2 output
    # pixels per cycle) — experimental, fp8 support unverified
    single_mode = mybir.MatmulPerfMode.DoublePixel if variant == "P" else None
    wap = wb2[:]
    wstride = list(wap.ap[0])
    for g, (taps, is_pair) in enumerate(groups):
        t = taps[0]
        if is_pair:
            lhsT = AP(wap.tensor, t * P, [wstride, [P, 2], [1, P]])
        else:
            lhsT = AP(wap.tensor, t * P, [wstride, [1, P]])
        # pair stride: distance between the two taps' windows (0 for the
        # duplicated tap-8 pair)
        pd = None
        if is_pair:
            pd = (OFF[taps[1]] - OFF[t]) if taps[1] <= 8 else 0
        for ps, r0 in zip(ps_list, r0_list):
            tg = min(t, 8)  # slot 9 is geometrically tap 8
            kh, kw = (tg // 3), (tg % 3)
            base = (r0 + kh) * RS + kw
            rhs = _rhs_ap(T, base, pd)
            nc.tensor.matmul(
                ps[:],
                lhsT,
                rhs,
                start=(g == 0),
                stop=(g == len(groups) - 1),
                perf_mode=dr if is_pair else single_mode,
            )


def build(n_imgs=IMGS, variant=VARIANT, n_cores=N_CORES):
    nc = bacc.Bacc(
        "TRN2", target_bir_lowering=False, debug=False, num_devices=n_cores
    )
    x_ext = nc.declare_dram_parameter("x", [n_imgs, P, H, W], F8, isOutput=False)
    wt_ext = nc.declare_dram_parameter("wt", [P, 10, P], F32, isOutput=False)
    b_ext = nc.declare_dram_parameter("b", [P, 1], F32, isOutput=False)
    out_ext = nc.declare_dram_parameter("out", [n_imgs, P, H, W], F16, isOutput=True)

    with tile.TileContext(nc) as tc:
        with (
            tc.tile_pool(name="wpool", bufs=1) as wpool,
            tc.tile_pool(name="inpool", bufs=4) as inpool,
            tc.tile_pool(name="tpool", bufs=4) as tpool,
            tc.tile_pool(name="outpool", bufs=5) as outpool,
            tc.tile_pool(name="pspool", bufs=8, space="PSUM") as pspool,
        ):
            # ---- weights / bias prep (one-time; DMA'd on the scalar ring so
            # the x chunks own the sync ring) ----
            # dependency-free DVE warmup: pays the first-instruction fetch +
            # ALU pipe warm-up at t~0 instead of in front of the first
            # binarize / weight-binarize
            # scratch doubles as DVE warm-up target and dummy-matmul operand
            scratch = wpool.tile([P, NTILE], F8)
            nc.vector.memset(scratch[:], 0.0)
            warm = wpool.tile([P, 16], F8)
            nc.vector.tensor_scalar(
                warm[:, 4:8], warm[:, 0:4], 0.0, 0.5, ALU.is_ge, ALU.subtract
            )
            # warm the strided-write path too (the first row-strided
            # binarize otherwise pays ~2.7us of pipe setup)
            wdst = AP(warm[:].tensor, 8, [list(warm[:].ap[0]), [2, 4]])
            nc.vector.tensor_scalar(
                wdst, warm[:, 0:4], 0.0, 0.5, ALU.is_ge, ALU.subtract
            )
            # HAM warm-up: ~3.5us of dummy matmuls on scratch data while the
            # tensor engine would otherwise idle waiting for the first input
            # chunk. The PE clock un-throttles (1.2 -> 2.4 GHz) after ~3.4us
            # of continuous activity, so the REAL stream starts at full
            # speed instead of paying the cold-clock penalty.
            ps_warm = pspool.tile([P, NTILE], F32, name="pswarm", tag="ps")
            for wi in range(9):
                nc.tensor.matmul(
                    ps_warm[:],
                    scratch[:, 0:P],
                    scratch[:],
                    start=(wi == 0),
                    stop=(wi == 8),
                )
            # weight transfer: taps 0-1 ride the sync ring (behind only the
            # 6-row chunk 0), the other 8 taps ride the scalar ring first —
            # both land ~3us after dispatch, just before the stream wants
            # weight-set 1
            wt_stage = wpool.tile([P, 10 * P], F32)
            nc.scalar.dma_start(wt_stage[:, 2 * P :], wt_ext[:, 2:10, :])
            bias = wpool.tile([P, 1], F32)
            nc.scalar.dma_start(bias[:], b_ext[:])
            # weights binarize straight to {-0.5, +0.5}; together with the
            # {-0.5, +0.5} activations every product is +-0.25, and the x4
            # rescale rides for free on the ACT drain (scale=4). One DVE op
            # per weight stage instead of two.
            whalf = wpool.tile([P, 10 * P], F8)  # {-0.5, +0.5}

            def emit_wstage(lo, hi, eng=None):
                # weight binarize rides the ACT engine (idle in the head;
                # the DVE is busy with the first input binarize): Sign(w)
                # gives {-1,+1} exactly (W has no exact zeros), so products
                # are +-0.5 and the drain rescale becomes x2
                nc.scalar.activation(
                    whalf[:, lo:hi], wt_stage[:, lo:hi], ACTF.Sign
                )

            Ts = {}

            def emit_borders(img):
                T = tpool.tile([P, TSIZE], F8)
                Ts[img] = T
                eng = nc.gpsimd
                if img == 0:
                    # the first matmul tiles only read border rows 0-35;
                    # zero those first so they don't gate the first real
                    # matmul behind the full-height strided memsets
                    eng.memset(T[:, 0:RS], 0.0)  # top zero row
                    eng.memset(T[:, 0 : 36 * RS : RS], 0.0)  # left rows 0-35
                    eng.memset(T[:, RS - 1 : 36 * RS : RS], 0.0)  # right
                    eng.memset(T[:, TSIZE - RS : TSIZE], 0.0)  # bottom
                    eng.memset(T[:, 36 * RS : TSIZE - RS + 1 : RS], 0.0)
                    eng.memset(T[:, 36 * RS + RS - 1 : TSIZE : RS], 0.0)
                else:
                    eng.memset(T[:, 0:RS], 0.0)  # top zero row
                    eng.memset(T[:, TSIZE - RS : TSIZE], 0.0)  # bottom
                    eng.memset(T[:, 0 : TSIZE - RS + 1 : RS], 0.0)  # left
                    eng.memset(T[:, RS - 1 : TSIZE : RS], 0.0)  # right

            def emit_chunks(img, splits):
                T = Ts[img]
                for r_lo, r_hi in zip(splits, splits[1:]):
                    nrows = r_hi - r_lo
                    xin = inpool.tile([P, QROWS * W], F8, name="xin", tag="xin")
                    # sync engine does nothing else -> input DMA dispatch is
                    # never gated behind compute in an engine FIFO
                    nc.sync.dma_start(
                        xin[:, : nrows * W], x_ext[img, :, r_lo:r_hi, :]
                    )
                    dst = AP(
                        T[:].tensor,
                        (r_lo + 1) * RS + 1,
                        [list(T[:].ap[0]), [RS, nrows], [1, W]],
                    )
                    nc.vector.tensor_scalar(
                        dst, xin[:, : nrows * W], 0.0, 0.5, ALU.is_ge, ALU.subtract
                    )

            # image 0: a 10-row leading chunk so the first (2-tile) matmul
            # group unblocks asap; the remaining weight slots binarize right
            # after it (gpsimd is ~15ns/elem at tensor_scalar — DVE only)
            emit_borders(0)
            # taps 0-1 lead the sync ring so their binarize (ACT) finishes
            # before the first input chunk's binarize (DVE) does
            nc.sync.dma_start(wt_stage[:, : 2 * P], wt_ext[:, 0:2, :])
            emit_wstage(0, 2 * P)
            emit_wstage(2 * P, 10 * P)
            emit_chunks(0, [0, 6, 34, 62, 90, 112])

            def emit_quarters(img):
                T = Ts[img]
                last_img = img == n_imgs - 1
                for q in range(4):
                    outsb = outpool.tile([P, CHUNK], F16)
                    # image 0 quarter 0 starts with 2-tile groups: their
                    # matmuls only need the first chunks + weight stage 1,
                    # and they warm the PE while the rest of the input lands.
                    # The very last quarter tapers so the final drains
                    # overlap the final matmuls instead of trailing them.
                    if img == 0 and q == 0:
                        sgroups = [1, 1, 2, 3]
                    elif last_img and q == 3:
                        sgroups = [4, 2, 1]
                    else:
                        sgroups = [4, 3]
                    s0 = 0
                    for glen in sgroups:
                        snames = list(range(s0, s0 + glen))
                        s0 += glen
                        ps_list = [
                            pspool.tile([P, NTILE], F32, name=f"ps{i}", tag="ps")
                            for i in range(len(snames))
                        ]
                        r0_list = [q * QROWS + s * TROWS for s in snames]
                        _emit_main_matmuls(nc, ps_list, whalf, T, r0_list, variant)
                        for gi, (ps, s) in enumerate(zip(ps_list, snames)):
                            if last_img and q == 3 and gi % 2 == 1:
                                # end-of-kernel: DVE has no binarizes left,
                                # so split the final drains across ACT+DVE
                                # (out = psum*4 + bias)
                                nc.vector.tensor_scalar(
                                    outsb[:, s * NTILE : (s + 1) * NTILE],
                                    ps[:],
                                    2.0,
                                    bias[:],
                                    ALU.mult,
                                    ALU.add,
                                )
                            else:
                                nc.scalar.activation(
                                    outsb[:, s * NTILE : (s + 1) * NTILE],
                                    ps[:],
                                    ACTF.Identity,
                                    bias=bias[:],
                                    scale=2.0,
                                )
                    # output doorbells ride on gpsimd so the scalar queue
                    # holds only ACTIVATEs; finer split on the very last
                    # quarter to shorten the kernel tail
                    # last-quarter pieces align with the [4, 2, 1] group
                    # taper so only the final 4 rows wait for the last drain
                    rsplits = [0, 16, 24, 28] if (last_img and q == 3) else [0, 16, 28]
                    for a, b2_ in zip(rsplits, rsplits[1:]):
                        nc.gpsimd.dma_start(
                            out_ext[img, :, q * QROWS + a : q * QROWS + b2_, :],
                            outsb[:, a * W : b2_ * W],
                        )

            for img in range(n_imgs):
                if img + 1 < n_imgs:
                    emit_borders(img + 1)
                    emit_chunks(img + 1, [0, 28, 56, 84, 112])
                emit_quarters(img)

    nc.compile()
    return nc


def _host_prep(x, W_, b):
    x = np.asarray(x, dtype=np.float32)
    W_ = np.asarray(W_, dtype=np.float32)
    b = np.asarray(b, dtype=np.float32)
    # [C_out, C_in, 3, 3] -> [C_in, tap, C_out]; tap 8 duplicated into a
    # 10th slot for the all-DoubleRow matmul schedule (pure layout change)
    wt9 = np.transpose(W_, (1, 2, 3, 0)).reshape(P, 9, P)
    wt = np.ascontiguousarray(
        np.concatenate([wt9, wt9[:, 8:9, :]], axis=1)
    )  # [P, 10, P]
    b2 = np.ascontiguousarray(b.reshape(P, 1))
    x8 = np.ascontiguousarray(x).astype(ml_dtypes.float8_e4m3)
    # negatives in (-2^-10, 0) round to -0.0 (byte 0x80), which device-side
    # is_ge(0) would binarize to +1 while the fp32 reference gives -1. Remap
    # -0.0 to the smallest negative normal (-2^-6, byte 0x88): binarization
    # only reads the sign, so this makes the fp8 transfer sign-exact.
    v = x8.view(np.uint8)
    v[v == 0x80] = 0x88
    return x8, wt, b2


def run(x, W, b, trace=False, variant=VARIANT, trace_cores=None):
    x8, wt, b2 = _host_prep(x, W, b)
    n = x8.shape[0]
    per = n // N_CORES
    nc = build(n_imgs=per, variant=variant)
    in_maps = [
        {"x": np.ascontiguousarray(x8[k * per : (k + 1) * per]), "wt": wt, "b": b2}
        for k in range(N_CORES)
    ]
    kwargs = {"trace_cores": trace_cores} if trace_cores else {}
    res = run_bass_kernel_spmd(nc, in_maps, list(range(N_CORES)), trace=trace, **kwargs)
    out = np.concatenate(
        [res.results[k]["out"].astype(np.float32) for k in range(N_CORES)], axis=0
    )
    return out, res


def kernel(x, W, b):
    out, _ = run(x, W, b, trace=False)
    return out


if __name__ == "__main__":
    xs = np.random.randn(32, P, H, W).astype(np.float32)
    Ws = np.random.randn(P, P, 3, 3).astype(np.float32) * 0.03
    bs = np.random.randn(P).astype(np.float32) * 0.01
    out = kernel(xs, Ws, bs)
    print(out.shape, out.dtype)
